# revision 5
# baseline (speedup 1.0000x reference)
"""Trainium2 Bass kernel for nn_Gtu2d (2-D gated Toeplitz unit).

Math: for each (head h, channel d) the length-2n circular FFT conv pair is an
exact product of two 128x128 Toeplitz matrices:
    o1 = T2 @ M @ T1.T = o2         (matmul associativity; both orders equal)
    mix = o1 + o2 = 2 * T2 @ M @ T1.T
with T[k, j] = rpe(k - j)[c].  The tiny RPE MLP depends only on the params
(t1/t2), so its 255-entry coefficient table per channel is computed on host;
the Toeplitz operand tiles are expanded on device by a sliding-window DMA.

Sharding: 8 cores = 4 batches x 2 head-groups (4 heads / 192 channels each).
Each core computes a partial (128, 16384) output in [d_model, token] layout
(tokens W-major); the host sums the two head-group partials per batch,
reorders tokens, and adds o_b.
"""

import numpy as np
import ml_dtypes
from contextlib import ExitStack

BF16 = ml_dtypes.bfloat16

B, N, DM = 4, 16384, 128
NH, HD = 8, 48
D1 = NH * HD                 # 384
HS = WS = 128                # spatial H, W (N = HS*WS)
RPE_DIM, RPE_LAYERS, EPS = 64, 3, 1e-5
N_CORES = 8
HG = 2                       # head groups (cores per batch)
C_LOC = (NH // HG) * HD      # 192 channels per core
CW = 2 * C_LOC               # v+u projection width per core (384)
TJ = 512                     # final-projection token chunk


def _ln(x, g, b):
    mu = x.mean(-1, keepdims=True)
    var = ((x - mu) ** 2).mean(-1, keepdims=True)
    return (x - mu) / np.sqrt(var + EPS) * g + b


def _rpe(p, idx):
    h = idx @ p["pos_W"].T + p["pos_b"]
    for i in range(RPE_LAYERS):
        z = np.maximum(_ln(h, p["l%d_g" % i], p["l%d_be" % i]), 0.0)
        h = z @ p["l%d_W" % i].T + p["l%d_b" % i]
    z = np.maximum(_ln(h, p["out_g"], p["out_be"]), 0.0)
    return z @ p["out_W"].T + p["out_b"]


def _slide_table(t):
    """(255, 384) table: vec[i, c] = rpe(i - 127)[c] so that
    T[k, j][c] = vec[k - j + 127, c]."""
    p = {k: np.asarray(v, np.float64) for k, v in t.items()}
    n = HS
    zero = _rpe(p, np.zeros((1, 1)))
    pos = _rpe(p, np.arange(1, n, dtype=np.float64)[:, None])
    neg = _rpe(p, -np.arange(n - 1, 0, -1, dtype=np.float64)[:, None])
    a = np.concatenate([zero, pos, zero, neg], 0)      # (256, 384), a[q] = rpe(q mod- 256)
    return np.concatenate([a[n + 1:], a[:n]], 0)       # (255, 384)


_PROG = None


def _build():
    import concourse.bass as bass
    import concourse.tile as tile
    from concourse import bacc, mybir

    bf16 = mybir.dt.bfloat16
    f32 = mybir.dt.float32
    Silu = mybir.ActivationFunctionType.Silu
    Copy = mybir.ActivationFunctionType.Copy

    nc = bacc.Bacc("TRN2", target_bir_lowering=False, debug=False,
                   enable_asserts=True, num_devices=N_CORES)

    xb = nc.dram_tensor("xb", [N, DM], bf16, kind="ExternalInput")
    wcat = nc.dram_tensor("wcat", [DM, CW], bf16, kind="ExternalInput")
    biascat = nc.dram_tensor("biascat", [1, CW], bf16, kind="ExternalInput")
    ones1 = nc.dram_tensor("ones1", [1, DM], bf16, kind="ExternalInput")
    rv1 = nc.dram_tensor("rv1", [C_LOC, WS, WS], bf16, kind="ExternalInput")
    rv2 = nc.dram_tensor("rv2", [C_LOC, WS, WS], bf16, kind="ExternalInput")
    owta = nc.dram_tensor("owta", [C_LOC // 2, DM], bf16, kind="ExternalInput")
    owtb = nc.dram_tensor("owtb", [C_LOC // 2, DM], bf16, kind="ExternalInput")
    outp = nc.dram_tensor("outp", [DM, N], f32, kind="ExternalOutput")

    with tile.TileContext(nc) as tc, ExitStack() as ctx:
        const = ctx.enter_context(tc.tile_pool(name="const", bufs=1))
        uvp = ctx.enter_context(tc.tile_pool(name="uv", bufs=1))
        gp = ctx.enter_context(tc.tile_pool(name="g", bufs=1))
        xtp = ctx.enter_context(tc.tile_pool(name="xt", bufs=4))
        rp = ctx.enter_context(tc.tile_pool(name="r", bufs=8))
        asbp = ctx.enter_context(tc.tile_pool(name="asb", bufs=4))
        gsbp = ctx.enter_context(tc.tile_pool(name="gsb", bufs=4))
        ostp = ctx.enter_context(tc.tile_pool(name="ost", bufs=3))
        ps_big = ctx.enter_context(tc.tile_pool(name="psb", bufs=3, space="PSUM"))
        ps_a = ctx.enter_context(tc.tile_pool(name="psa", bufs=2, space="PSUM"))
        ps_z = ctx.enter_context(tc.tile_pool(name="psz", bufs=2, space="PSUM"))

        wcat_sb = const.tile([DM, CW], bf16, tag="wcat")
        nc.sync.dma_start(out=wcat_sb[:, :], in_=wcat.ap())
        bias_sb = const.tile([1, CW], bf16, tag="bias")
        nc.sync.dma_start(out=bias_sb[:, :], in_=biascat.ap())
        ones_sb = const.tile([1, DM], bf16, tag="ones")
        nc.sync.dma_start(out=ones_sb[:, :], in_=ones1.ap())
        owta_sb = const.tile([C_LOC // 2, DM], bf16, tag="owta")
        nc.sync.dma_start(out=owta_sb[:, :], in_=owta.ap())
        owtb_sb = const.tile([C_LOC // 2, DM], bf16, tag="owtb")
        nc.sync.dma_start(out=owtb_sb[:, :], in_=owtb.ap())

        # UV[w, h*CW + c]: c in [0,192) = silu(v), c in [192,384) = silu(u)
        UV = uvp.tile([WS, HS * CW], bf16, tag="uv")
        gA = gp.tile([C_LOC // 2, N], bf16, tag="ga")   # rows c=0..95, t' = w*128+h
        gB = gp.tile([C_LOC // 2, N], bf16, tag="gb")   # rows c=96..191

        # ---- Phase A: projections.  Tile i = tokens (H=i, W=0..127). ----
        for i in range(HS):
            xt = xtp.tile([DM, WS], bf16, tag="xt")
            nc.sync.dma_start(out=xt[:, :], in_=xb.ap()[i * WS:(i + 1) * WS, :],
                              transpose=True)
            pu = ps_big.tile([WS, CW], f32, tag="pu")
            nc.tensor.matmul(pu[:, :], lhsT=xt[:, :], rhs=wcat_sb[:, :],
                             start=True, stop=False)
            nc.tensor.matmul(pu[:, :], lhsT=ones_sb[:, :], rhs=bias_sb[:, :],
                             start=False, stop=True)
            nc.scalar.activation(UV[:, i * CW:(i + 1) * CW], pu[:, :], Silu)

        # ---- Phase B: per-(head,channel) Toeplitz mixing + gating. ----
        uv3 = UV[:, :].rearrange("p (h c) -> p h c", c=CW)
        for hd in range(C_LOC):
            r1 = rp.tile([WS, WS], bf16, tag="r")
            nc.sync.dma_start(out=r1[:, :], in_=rv1.ap()[hd, :, :])
            r2 = rp.tile([WS, WS], bf16, tag="r")
            nc.sync.dma_start(out=r2[:, :], in_=rv2.ap()[hd, :, :])
            # A = M @ T1.T   (lhsT = M.T as a stride-CW view of UV)
            pa = ps_a.tile([HS, WS], f32, tag="pa")
            nc.tensor.matmul(pa[:, :], lhsT=uv3[:, :, hd], rhs=r1[:, :],
                             start=True, stop=True)
            asb = asbp.tile([HS, WS], bf16, tag="asb")
            nc.scalar.activation(asb[:, :], pa[:, :], Copy)
            # ZT = (T2 @ A).T = mix.T  (x2 folded into rv2)
            pz = ps_z.tile([WS, HS], f32, tag="pz")
            nc.tensor.matmul(pz[:, :], lhsT=asb[:, :], rhs=r2[:, :],
                             start=True, stop=True)
            gsb = gsbp.tile([WS, HS], bf16, tag="gsb")
            nc.vector.tensor_mul(gsb[:, :], pz[:, :], uv3[:, :, C_LOC + hd])
            dst = gA if hd < C_LOC // 2 else gB
            row = hd % (C_LOC // 2)
            nc.sync.dma_start(
                out=dst[row:row + 1, :].rearrange("p (w h) -> p w h", w=WS),
                in_=gsb[:, :])

        # ---- Phase C: output projection, accumulated over the 2 c-chunks. ----
        for j in range(N // TJ):
            po = ps_big.tile([DM, TJ], f32, tag="pu")
            nc.tensor.matmul(po[:, :], lhsT=owta_sb[:, :],
                             rhs=gA[:, j * TJ:(j + 1) * TJ], start=True, stop=False)
            nc.tensor.matmul(po[:, :], lhsT=owtb_sb[:, :],
                             rhs=gB[:, j * TJ:(j + 1) * TJ], start=False, stop=True)
            ost = ostp.tile([DM, TJ], f32, tag="ost")
            nc.vector.tensor_copy(ost[:, :], po[:, :])
            nc.sync.dma_start(out=outp.ap()[:, j * TJ:(j + 1) * TJ], in_=ost[:, :])

    nc.compile()
    return nc


def _get_prog():
    global _PROG
    if _PROG is None:
        _PROG = _build()
    return _PROG


def core_inputs(x, u_W, u_b, v_W, v_b, o_W, o_b, t1, t2):
    """Per-core input maps (host-side shard + param prep)."""
    x = np.asarray(x)
    u_W, u_b = np.asarray(u_W), np.asarray(u_b)
    v_W, v_b = np.asarray(v_W), np.asarray(v_b)
    o_W = np.asarray(o_W)
    vec1 = _slide_table(t1)
    vec2 = _slide_table(t2) * 2.0          # fold mix = 2 * T2 M T1.T
    # win[j, k, c] = vec[127 - j + k, c]  (the Toeplitz operand tiles)
    win1 = np.stack([vec1[127 - j:255 - j] for j in range(WS)], axis=0)
    win2 = np.stack([vec2[127 - j:255 - j] for j in range(WS)], axis=0)
    ones_np = np.ones((1, DM), BF16)
    in_maps = []
    for core in range(N_CORES):
        b, hg = divmod(core, HG)
        c0 = hg * C_LOC
        wc = np.concatenate([v_W[c0:c0 + C_LOC, :].T, u_W[c0:c0 + C_LOC, :].T], axis=1)
        bc = np.concatenate([v_b[c0:c0 + C_LOC], u_b[c0:c0 + C_LOC]])[None, :]
        in_maps.append({
            "xb": x[b].astype(BF16),
            "wcat": np.ascontiguousarray(wc).astype(BF16),
            "biascat": np.ascontiguousarray(bc).astype(BF16),
            "ones1": ones_np,
            "rv1": np.ascontiguousarray(win1[:, :, c0:c0 + C_LOC].transpose(2, 0, 1)).astype(BF16),
            "rv2": np.ascontiguousarray(win2[:, :, c0:c0 + C_LOC].transpose(2, 0, 1)).astype(BF16),
            "owta": np.ascontiguousarray(o_W[:, c0:c0 + 96].T).astype(BF16),
            "owtb": np.ascontiguousarray(o_W[:, c0 + 96:c0 + C_LOC].T).astype(BF16),
        })
    return in_maps


def assemble(per_core_outs, o_b):
    """Sum head-group partials, undo W-major token order, add o_b."""
    o_b = np.asarray(o_b, np.float32)
    out = np.empty((B, N, DM), np.float32)
    for b in range(B):
        p = per_core_outs[b * HG]["outp"] + per_core_outs[b * HG + 1]["outp"]
        out[b] = p.reshape(DM, WS, HS).transpose(2, 1, 0).reshape(N, DM)
    out += o_b[None, None, :]
    return out


def kernel(x, u_W, u_b, v_W, v_b, o_W, o_b, t1, t2):
    from concourse.bass_utils import run_bass_kernel_spmd
    nc = _get_prog()
    in_maps = core_inputs(x, u_W, u_b, v_W, v_b, o_W, o_b, t1, t2)
    res = run_bass_kernel_spmd(nc, in_maps, core_ids=list(range(N_CORES)))
    return assemble(res.results, o_b)
